# revision 21
# baseline (speedup 1.0000x reference)
"""Trainium2 Bass kernel for nn_AttentionMatrix.

Computes, for mat_0:[B,N,H], mat_1:[B,M,H], w:[3H], bias:[1]:
    out[b,n,m] = sum_h mat_0[b,n,h]*w2[h]*mat_1[b,m,h] + s0[b,n] + s1[b,m] + C
with s0 = mat_0@w0, s1 = mat_1@w1, C = bias[0].

Strategy: data-parallel over batch across 8 NeuronCores (2 batches/core).
Host-side prep (layout only + the 0.1%-of-FLOPs rank-1 vectors):
  - a_t = (mat_0 * w2)^T per batch, cast bf16  -> [bpc, H, N]
  - b_t = mat_1^T per batch, cast bf16         -> [bpc, H, M]
  - s0 as [P, bpc*nt] column tiles; s1 = mat_1@w1 + C as [1, bpc*M] rows
    (broadcast to 128 partitions on-chip by the idle Pool engine).
Device: pure-GEMM mains psum[128n, 1024m] += a_k[h,n].T @ b_k[h,m] in bf16
(1 cycle/row on the PE array — no on-chip transposes), fused DVE epilogue
(psum + s0_col + s1_row -> bf16), bf16 stores. Host upconverts to f32.

bf16 I/O halves DMA traffic (25 MB/core vs 50) and removes the 2-cycle/row
f32 transpose tax, leaving the kernel at the PE matmul roofline. Input
loads issue half-width on the Activation HWDGE queue (finer supply at the
pipeline head) and stores on the SP queue; psum runs 4x[128,1024] banks
for eviction ILP; the final tile drains in 512-wide chunks to shorten
the tail chain.
"""

import numpy as np

import concourse.bacc as bacc
import concourse.bass as bass
import concourse.mybir as mybir
from concourse.tile import TileContext

F32 = mybir.dt.float32
BF16 = mybir.dt.bfloat16
ADD = mybir.AluOpType.add

P = 128

# Problem dims (hardcoded per contract)
B, N, M, H = 16, 2048, 2048, 512
N_CORES = 8
BPC = B // N_CORES  # batches per core


def build_program(bpc=BPC, n=N, m=M, h=H):
    kt = h // P        # contraction k-tiles (4)
    nt = n // P        # n-tiles (16)
    hw = n // 2        # half-stripe width (1024)

    nc = bacc.Bacc("TRN2", target_bir_lowering=False, debug=False)
    a_t = nc.dram_tensor("a_t", [bpc, h, n], BF16, kind="ExternalInput").ap()
    b_t = nc.dram_tensor("b_t", [bpc, h, m], BF16, kind="ExternalInput").ap()
    s0a = nc.dram_tensor("s0a", [P, bpc * nt], F32, kind="ExternalInput").ap()
    s1r = nc.dram_tensor("s1r", [1, bpc * m], F32, kind="ExternalInput").ap()
    # bf16 single-row copies of s0/s1 for the rank-1 matmul fold used by the
    # final drain chunk
    s0rb = nc.dram_tensor("s0rb", [1, bpc * n], BF16, kind="ExternalInput").ap()
    s1rb = nc.dram_tensor("s1rb", [1, bpc * m], BF16, kind="ExternalInput").ap()
    out = nc.dram_tensor("out", [bpc, n, m], BF16, kind="ExternalOutput").ap()

    with TileContext(nc) as tc:
        with (
            tc.tile_pool(name="const", bufs=1) as cpool,
            tc.tile_pool(name="ops", bufs=2) as ops,
            tc.tile_pool(name="vecs", bufs=2) as vpool,
            tc.tile_pool(name="ob", bufs=8) as obpool,
            tc.tile_pool(name="mpsum", bufs=4, space="PSUM") as mpsum,
        ):
            # PE p-state warm-up: the tensor engine ramps 0.65->1.2->2.4 GHz
            # over ~3us of continuous execution. Run throwaway matmuls on a
            # zeroed tile while the first operand stripes stream in, so real
            # matmuls start at full clock.
            warm = cpool.tile([P, 512], BF16)
            nc.gpsimd.memset(warm, 0.0)
            ones = cpool.tile([1, 512], BF16)
            nc.gpsimd.memset(ones, 1.0)
            wp = mpsum.tile([P, hw], F32, tag="mm", name="wp")
            for _ in range(7):
                nc.tensor.matmul(
                    wp[:, :512], lhsT=warm[:, :P], rhs=warm,
                    start=True, stop=True,
                )
            for _ in range(30):
                nc.tensor.matmul(
                    wp[:, :16], lhsT=warm[:, :P], rhs=warm[:, :16],
                    start=True, stop=True,
                )

            def emit_vecs():
                s0all = cpool.tile([P, bpc * nt], F32)
                nc.sync.dma_start(out=s0all, in_=s0a)
                s1rows = cpool.tile([1, bpc * m], F32)
                nc.sync.dma_start(out=s1rows, in_=s1r)
                s0row = cpool.tile([1, bpc * n], BF16)
                nc.sync.dma_start(out=s0row, in_=s0rb)
                s1row = cpool.tile([1, bpc * m], BF16)
                nc.sync.dma_start(out=s1row, in_=s1rb)
                vpool.rank1 = (s0row, s1row)
                s1bs = []
                for bi in range(bpc):
                    s1b = vpool.tile([P, m], F32, tag=f"s1b{bi}",
                                     name=f"s1b{bi}")
                    nc.gpsimd.partition_broadcast(
                        s1b, s1rows[0:1, bi * m:(bi + 1) * m]
                    )
                    s1bs.append(s1b)
                return s0all, s1bs

            def emit_loads(bi, vecs_after_k=None):
                # supply-ordered loads: the PE k-loop needs b/a h0 of every
                # k quickly, plus b h1 (tiles 0-7 span chunks 0-3); a h1
                # feeds tiles 8-15 and can arrive last
                ak = [[None, None] for _ in range(kt)]
                bk = [[None, None] for _ in range(kt)]
                vecs = None

                def load(arr, dst, k, j, tag):
                    t = ops.tile([P, hw], BF16, tag=f"{tag}{k}h{j}",
                                 name=f"{tag}{k}h{j}")
                    nc.scalar.dma_start(
                        out=t,
                        in_=arr[bi, k * P:(k + 1) * P, j * hw:(j + 1) * hw],
                    )
                    dst[k][j] = t

                for k in range(kt):
                    load(b_t, bk, k, 0, "b")
                    load(a_t, ak, k, 0, "a")
                    load(b_t, bk, k, 1, "b")
                    if vecs_after_k == k:
                        vecs = emit_vecs()
                for k in range(kt):
                    load(a_t, ak, k, 1, "a")
                if vecs is not None:
                    return ak, bk, vecs
                return ak, bk

            def lhs(ak, k, t):
                return ak[k][t // 8][:, (t % 8) * P:(t % 8 + 1) * P]

            def rhs(bk, k, c):
                # c indexes 512-wide chunks (0..3)
                return bk[k][c // 2][:, (c % 2) * 512:(c % 2 + 1) * 512]

            def emit_mains(bi, ak, bk, s0all, s1bs, last=False):
                s0c = s0all[:, bi * nt:(bi + 1) * nt]
                s1b = s1bs[bi]
                for t in range(nt):
                    for half in range(2):
                        if last and t == nt - 1 and half == 1:
                            # final half-tile: drain in shrinking chunks.
                            # The last chunk folds the rank-1 epilogue into
                            # two K=1 matmuls so its eviction is a plain ACT
                            # copy and its store rides the idle Pool SWDGE
                            # queue — the shortest possible tail chain.
                            s0row, s1row = vpool.rank1
                            drains = ((1024, 512, nc.scalar, False),
                                      (1536, 256, nc.sync, False),
                                      (1792, 128, nc.scalar, False),
                                      (1920, 128, nc.gpsimd, True))
                            for c0, cw, eng, fold in drains:
                                mp = mpsum.tile([P, hw], F32, tag="mm",
                                                name="mp")
                                for k in range(kt):
                                    nc.tensor.matmul(
                                        mp[:, :cw],
                                        lhsT=lhs(ak, k, t),
                                        rhs=bk[k][1][:, c0 - hw:c0 - hw + cw],
                                        start=(k == 0),
                                        stop=(k == kt - 1) and not fold,
                                    )
                                ob = obpool.tile([P, 512], BF16, tag="obl",
                                                 name="obl")
                                if fold:
                                    nc.tensor.matmul(
                                        mp[:, :cw],
                                        lhsT=s0row[0:1,
                                                   bi * n + t * P:
                                                   bi * n + (t + 1) * P],
                                        rhs=ones[0:1, :cw],
                                        start=False, stop=False,
                                    )
                                    nc.tensor.matmul(
                                        mp[:, :cw],
                                        lhsT=ones[0:1, :P],
                                        rhs=s1row[0:1,
                                                  bi * m + c0:
                                                  bi * m + c0 + cw],
                                        start=False, stop=True,
                                    )
                                    nc.scalar.copy(
                                        out=ob[:, :cw], in_=mp[:, :cw]
                                    )
                                else:
                                    nc.vector.scalar_tensor_tensor(
                                        out=ob[:, :cw],
                                        in0=mp[:, :cw],
                                        scalar=s0c[:, t:t + 1],
                                        in1=s1b[:, c0:c0 + cw],
                                        op0=ADD,
                                        op1=ADD,
                                    )
                                eng.dma_start(
                                    out=out[bi, t * P:(t + 1) * P,
                                            c0:c0 + cw],
                                    in_=ob[:, :cw],
                                )
                            continue
                        mp = mpsum.tile([P, hw], F32, tag="mm", name="mp")
                        for k in range(kt):
                            for cc in range(2):
                                c = half * 2 + cc
                                nc.tensor.matmul(
                                    mp[:, cc * 512:(cc + 1) * 512],
                                    lhsT=lhs(ak, k, t),
                                    rhs=rhs(bk, k, c),
                                    start=(k == 0),
                                    stop=(k == kt - 1),
                                )
                        ob = obpool.tile([P, hw], BF16, tag="ob", name="ob")
                        nc.vector.scalar_tensor_tensor(
                            out=ob,
                            in0=mp,
                            scalar=s0c[:, t:t + 1],
                            in1=s1b[:, half * hw:(half + 1) * hw],
                            op0=ADD,
                            op1=ADD,
                        )
                        seng = nc.sync if (t * 2 + half) % 2 == 0 else nc.scalar
                        seng.dma_start(
                            out=out[bi, t * P:(t + 1) * P,
                                    half * hw:(half + 1) * hw],
                            in_=ob,
                        )

            ak0, bk0, (s0all, s1bs) = emit_loads(0, vecs_after_k=1)
            la = (ak0, bk0)
            for bi in range(1, bpc):
                la_next = emit_loads(bi)
                emit_mains(bi - 1, *la, s0all, s1bs)
                la = la_next
            emit_mains(bpc - 1, *la, s0all, s1bs, last=True)
    nc.compile()
    return nc


_CACHE = {}


def _get_program():
    if "nc" not in _CACHE:
        _CACHE["nc"] = build_program()
    return _CACHE["nc"]


def make_in_maps(inputs, bpc=BPC, n_cores=N_CORES, n=N, m=M, h=H):
    import ml_dtypes

    bf16 = ml_dtypes.bfloat16
    mat_0 = np.asarray(inputs["mat_0"], dtype=np.float32)
    mat_1 = np.asarray(inputs["mat_1"], dtype=np.float32)
    w = np.asarray(inputs["w"], dtype=np.float32)
    bias = np.asarray(inputs["bias"], dtype=np.float32)
    w0, w1, w2 = w[:h], w[h:2 * h], w[2 * h:]
    nt = n // P
    # host-side rank-1 epilogue vectors
    s0 = mat_0 @ w0                      # [B, n]
    s1 = mat_1 @ w1 + bias[0]            # [B, m]
    # layouts for direct DMA: pre-transposed bf16 operands
    a_t = np.ascontiguousarray(
        (mat_0 * w2).transpose(0, 2, 1)
    ).astype(bf16)                       # [B, h, n]
    b_t = np.ascontiguousarray(
        mat_1.transpose(0, 2, 1)
    ).astype(bf16)                       # [B, h, m]
    # s0 for core c: [P, bpc*nt] with batch-major columns
    s0t = np.ascontiguousarray(
        s0.reshape(-1, nt, P).transpose(0, 2, 1)              # [B, P, nt]
    )
    in_maps = []
    for c in range(n_cores):
        sl = slice(c * bpc, (c + 1) * bpc)
        s0a = np.ascontiguousarray(
            s0t[sl].transpose(1, 0, 2).reshape(P, bpc * nt)
        )
        in_maps.append(
            {
                "a_t": a_t[sl],
                "b_t": b_t[sl],
                "s0a": s0a,
                "s1r": np.ascontiguousarray(s1[sl].reshape(1, bpc * m)),
                "s0rb": np.ascontiguousarray(
                    s0[sl].reshape(1, bpc * n)).astype(bf16),
                "s1rb": np.ascontiguousarray(
                    s1[sl].reshape(1, bpc * m)).astype(bf16),
            }
        )
    return in_maps


def kernel(**inputs) -> np.ndarray:
    from concourse import bass_utils

    nc = _get_program()
    res = bass_utils.run_bass_kernel_spmd(
        nc, make_in_maps(inputs), core_ids=list(range(N_CORES))
    )
    return np.concatenate(
        [np.asarray(res.results[c]["out"]).astype(np.float32)
         for c in range(N_CORES)],
        axis=0,
    )


# revision 24
# speedup vs baseline: 1.0021x; 1.0021x over previous
"""Trainium2 Bass kernel for nn_AttentionMatrix.

Computes, for mat_0:[B,N,H], mat_1:[B,M,H], w:[3H], bias:[1]:
    out[b,n,m] = sum_h mat_0[b,n,h]*w2[h]*mat_1[b,m,h] + s0[b,n] + s1[b,m] + C
with s0 = mat_0@w0, s1 = mat_1@w1, C = bias[0].

Strategy: data-parallel over batch across 8 NeuronCores (2 batches/core).
Host-side prep (layout only + the 0.1%-of-FLOPs rank-1 vectors):
  - a_t = (mat_0 * w2)^T per batch, cast bf16  -> [bpc, H, N]
  - b_t = mat_1^T per batch, cast bf16         -> [bpc, H, M]
  - s0 as [P, bpc*nt] column tiles; s1 = mat_1@w1 + C as [1, bpc*M] rows
    (broadcast to 128 partitions on-chip by the idle Pool engine).
Device: pure-GEMM mains psum[128n, 1024m] += a_k[h,n].T @ b_k[h,m] in bf16
(1 cycle/row on the PE array — no on-chip transposes), fused DVE epilogue
(psum + s0_col + s1_row -> bf16), bf16 stores. Host upconverts to f32.

bf16 I/O halves DMA traffic (25 MB/core vs 50) and removes the 2-cycle/row
f32 transpose tax, leaving the kernel at the PE matmul roofline. Input
loads issue half-width on the Activation HWDGE queue (finer supply at the
pipeline head) and stores on the SP queue; psum runs 4x[128,1024] banks
for eviction ILP; the final tile drains in 512-wide chunks to shorten
the tail chain.
"""

import numpy as np

import concourse.bacc as bacc
import concourse.bass as bass
import concourse.mybir as mybir
from concourse.tile import TileContext

F32 = mybir.dt.float32
BF16 = mybir.dt.bfloat16
ADD = mybir.AluOpType.add

P = 128

# Problem dims (hardcoded per contract)
B, N, M, H = 16, 2048, 2048, 512
N_CORES = 8
BPC = B // N_CORES  # batches per core


def build_program(bpc=BPC, n=N, m=M, h=H):
    kt = h // P        # contraction k-tiles (4)
    nt = n // P        # n-tiles (16)
    hw = n // 2        # half-stripe width (1024)

    nc = bacc.Bacc("TRN2", target_bir_lowering=False, debug=False)
    a_t = nc.dram_tensor("a_t", [bpc, h, n], BF16, kind="ExternalInput").ap()
    b_t = nc.dram_tensor("b_t", [bpc, h, m], BF16, kind="ExternalInput").ap()
    s0a = nc.dram_tensor("s0a", [P, bpc * nt], F32, kind="ExternalInput").ap()
    s1r = nc.dram_tensor("s1r", [1, bpc * m], F32, kind="ExternalInput").ap()
    # bf16 single-row copies of s0/s1 for the rank-1 matmul fold used by the
    # final drain chunk
    s0rb = nc.dram_tensor("s0rb", [1, bpc * n], BF16, kind="ExternalInput").ap()
    s1rb = nc.dram_tensor("s1rb", [1, bpc * m], BF16, kind="ExternalInput").ap()
    out = nc.dram_tensor("out", [bpc, n, m], BF16, kind="ExternalOutput").ap()

    with TileContext(nc) as tc:
        with (
            tc.tile_pool(name="const", bufs=1) as cpool,
            tc.tile_pool(name="ops", bufs=2) as ops,
            tc.tile_pool(name="vecs", bufs=2) as vpool,
            tc.tile_pool(name="ob", bufs=8) as obpool,
            tc.tile_pool(name="mpsum", bufs=4, space="PSUM") as mpsum,
        ):
            # PE p-state warm-up: the tensor engine ramps 0.65->1.2->2.4 GHz
            # over ~3us of continuous execution. Run throwaway matmuls on a
            # zeroed tile while the first operand stripes stream in, so real
            # matmuls start at full clock.
            warm = cpool.tile([P, 512], BF16)
            nc.gpsimd.memset(warm, 0.0)
            ones = cpool.tile([1, 512], BF16)
            nc.gpsimd.memset(ones, 1.0)
            wp = mpsum.tile([P, hw], F32, tag="mm", name="wp")
            for _ in range(7):
                nc.tensor.matmul(
                    wp[:, :512], lhsT=warm[:, :P], rhs=warm,
                    start=True, stop=True,
                )
            for _ in range(30):
                nc.tensor.matmul(
                    wp[:, :16], lhsT=warm[:, :P], rhs=warm[:, :16],
                    start=True, stop=True,
                )

            def emit_vecs():
                s0all = cpool.tile([P, bpc * nt], F32)
                nc.sync.dma_start(out=s0all, in_=s0a)
                s1rows = cpool.tile([1, bpc * m], F32)
                nc.sync.dma_start(out=s1rows, in_=s1r)
                s0row = cpool.tile([1, bpc * n], BF16)
                nc.sync.dma_start(out=s0row, in_=s0rb)
                s1row = cpool.tile([1, bpc * m], BF16)
                nc.sync.dma_start(out=s1row, in_=s1rb)
                vpool.rank1 = (s0row, s1row)
                s1bs = []
                for bi in range(bpc):
                    s1b = vpool.tile([P, m], F32, tag=f"s1b{bi}",
                                     name=f"s1b{bi}")
                    nc.gpsimd.partition_broadcast(
                        s1b, s1rows[0:1, bi * m:(bi + 1) * m]
                    )
                    s1bs.append(s1b)
                return s0all, s1bs

            def emit_loads(bi, vecs_after_k=None):
                # supply-ordered loads: the PE k-loop needs b/a h0 of every
                # k quickly, plus b h1 (tiles 0-7 span chunks 0-3); a h1
                # feeds tiles 8-15 and can arrive last
                ak = [[None, None] for _ in range(kt)]
                bk = [[None, None] for _ in range(kt)]
                vecs = None

                def load(arr, dst, k, j, tag):
                    t = ops.tile([P, hw], BF16, tag=f"{tag}{k}h{j}",
                                 name=f"{tag}{k}h{j}")
                    nc.scalar.dma_start(
                        out=t,
                        in_=arr[bi, k * P:(k + 1) * P, j * hw:(j + 1) * hw],
                    )
                    dst[k][j] = t

                for k in range(kt):
                    load(b_t, bk, k, 0, "b")
                    load(a_t, ak, k, 0, "a")
                    load(b_t, bk, k, 1, "b")
                    if vecs_after_k == k:
                        vecs = emit_vecs()
                for k in range(kt):
                    load(a_t, ak, k, 1, "a")
                if vecs is not None:
                    return ak, bk, vecs
                return ak, bk

            def lhs(ak, k, t):
                return ak[k][t // 8][:, (t % 8) * P:(t % 8 + 1) * P]

            def rhs(bk, k, c):
                # c indexes 512-wide chunks (0..3)
                return bk[k][c // 2][:, (c % 2) * 512:(c % 2 + 1) * 512]

            def emit_mains(bi, ak, bk, s0all, s1bs, last=False):
                s0c = s0all[:, bi * nt:(bi + 1) * nt]
                s1b = s1bs[bi]
                # group tiles in fours, h0 halves before h1, so the first
                # in-flight psum set only needs the h0 stripes (earlier PE
                # start at the pipeline head)
                order = []
                for g in range(0, nt, 4):
                    order += [(t, 0) for t in range(g, g + 4)]
                    order += [(t, 1) for t in range(g, g + 4)]
                for oi, (t, half) in enumerate(order):
                    if True:
                        if last and t == nt - 1 and half == 1:
                            # final half-tile: drain in shrinking chunks.
                            # The last chunk folds the rank-1 epilogue into
                            # two K=1 matmuls so its eviction is a plain ACT
                            # copy and its store rides the idle Pool SWDGE
                            # queue — the shortest possible tail chain.
                            s0row, s1row = vpool.rank1
                            drains = ((1024, 512, nc.sync, False),
                                      (1536, 256, nc.sync, False),
                                      (1792, 128, nc.sync, False),
                                      (1920, 128, nc.gpsimd, True))
                            for c0, cw, eng, fold in drains:
                                mp = mpsum.tile([P, hw], F32, tag="mm",
                                                name="mp")
                                for k in range(kt):
                                    nc.tensor.matmul(
                                        mp[:, :cw],
                                        lhsT=lhs(ak, k, t),
                                        rhs=bk[k][1][:, c0 - hw:c0 - hw + cw],
                                        start=(k == 0),
                                        stop=(k == kt - 1) and not fold,
                                    )
                                ob = obpool.tile([P, 512], BF16, tag="obl",
                                                 name="obl")
                                if fold:
                                    nc.tensor.matmul(
                                        mp[:, :cw],
                                        lhsT=s0row[0:1,
                                                   bi * n + t * P:
                                                   bi * n + (t + 1) * P],
                                        rhs=ones[0:1, :cw],
                                        start=False, stop=False,
                                    )
                                    nc.tensor.matmul(
                                        mp[:, :cw],
                                        lhsT=ones[0:1, :P],
                                        rhs=s1row[0:1,
                                                  bi * m + c0:
                                                  bi * m + c0 + cw],
                                        start=False, stop=True,
                                    )
                                    nc.scalar.copy(
                                        out=ob[:, :cw], in_=mp[:, :cw]
                                    )
                                else:
                                    nc.vector.scalar_tensor_tensor(
                                        out=ob[:, :cw],
                                        in0=mp[:, :cw],
                                        scalar=s0c[:, t:t + 1],
                                        in1=s1b[:, c0:c0 + cw],
                                        op0=ADD,
                                        op1=ADD,
                                    )
                                eng.dma_start(
                                    out=out[bi, t * P:(t + 1) * P,
                                            c0:c0 + cw],
                                    in_=ob[:, :cw],
                                )
                            continue
                        mp = mpsum.tile([P, hw], F32, tag="mm", name="mp")
                        for k in range(kt):
                            for cc in range(2):
                                c = half * 2 + cc
                                nc.tensor.matmul(
                                    mp[:, cc * 512:(cc + 1) * 512],
                                    lhsT=lhs(ak, k, t),
                                    rhs=rhs(bk, k, c),
                                    start=(k == 0),
                                    stop=(k == kt - 1),
                                )
                        ob = obpool.tile([P, hw], BF16, tag="ob", name="ob")
                        nc.vector.scalar_tensor_tensor(
                            out=ob,
                            in0=mp,
                            scalar=s0c[:, t:t + 1],
                            in1=s1b[:, half * hw:(half + 1) * hw],
                            op0=ADD,
                            op1=ADD,
                        )
                        seng = nc.sync if oi % 2 == 0 else nc.scalar
                        seng.dma_start(
                            out=out[bi, t * P:(t + 1) * P,
                                    half * hw:(half + 1) * hw],
                            in_=ob,
                        )

            ak0, bk0, (s0all, s1bs) = emit_loads(0, vecs_after_k=1)
            la = (ak0, bk0)
            for bi in range(1, bpc):
                la_next = emit_loads(bi)
                emit_mains(bi - 1, *la, s0all, s1bs)
                la = la_next
            emit_mains(bpc - 1, *la, s0all, s1bs, last=True)
    nc.compile()
    return nc


_CACHE = {}


def _get_program():
    if "nc" not in _CACHE:
        _CACHE["nc"] = build_program()
    return _CACHE["nc"]


def make_in_maps(inputs, bpc=BPC, n_cores=N_CORES, n=N, m=M, h=H):
    import ml_dtypes

    bf16 = ml_dtypes.bfloat16
    mat_0 = np.asarray(inputs["mat_0"], dtype=np.float32)
    mat_1 = np.asarray(inputs["mat_1"], dtype=np.float32)
    w = np.asarray(inputs["w"], dtype=np.float32)
    bias = np.asarray(inputs["bias"], dtype=np.float32)
    w0, w1, w2 = w[:h], w[h:2 * h], w[2 * h:]
    nt = n // P
    # host-side rank-1 epilogue vectors
    s0 = mat_0 @ w0                      # [B, n]
    s1 = mat_1 @ w1 + bias[0]            # [B, m]
    # layouts for direct DMA: pre-transposed bf16 operands
    a_t = np.ascontiguousarray(
        (mat_0 * w2).transpose(0, 2, 1)
    ).astype(bf16)                       # [B, h, n]
    b_t = np.ascontiguousarray(
        mat_1.transpose(0, 2, 1)
    ).astype(bf16)                       # [B, h, m]
    # s0 for core c: [P, bpc*nt] with batch-major columns
    s0t = np.ascontiguousarray(
        s0.reshape(-1, nt, P).transpose(0, 2, 1)              # [B, P, nt]
    )
    in_maps = []
    for c in range(n_cores):
        sl = slice(c * bpc, (c + 1) * bpc)
        s0a = np.ascontiguousarray(
            s0t[sl].transpose(1, 0, 2).reshape(P, bpc * nt)
        )
        in_maps.append(
            {
                "a_t": a_t[sl],
                "b_t": b_t[sl],
                "s0a": s0a,
                "s1r": np.ascontiguousarray(s1[sl].reshape(1, bpc * m)),
                "s0rb": np.ascontiguousarray(
                    s0[sl].reshape(1, bpc * n)).astype(bf16),
                "s1rb": np.ascontiguousarray(
                    s1[sl].reshape(1, bpc * m)).astype(bf16),
            }
        )
    return in_maps


def kernel(**inputs) -> np.ndarray:
    from concourse import bass_utils

    nc = _get_program()
    res = bass_utils.run_bass_kernel_spmd(
        nc, make_in_maps(inputs), core_ids=list(range(N_CORES))
    )
    return np.concatenate(
        [np.asarray(res.results[c]["out"]).astype(np.float32)
         for c in range(N_CORES)],
        axis=0,
    )


# revision 28
# speedup vs baseline: 1.0123x; 1.0103x over previous
"""Trainium2 Bass kernel for nn_AttentionMatrix.

Computes, for mat_0:[B,N,H], mat_1:[B,M,H], w:[3H], bias:[1]:
    out[b,n,m] = sum_h mat_0[b,n,h]*w2[h]*mat_1[b,m,h] + s0[b,n] + s1[b,m] + C
with s0 = mat_0@w0, s1 = mat_1@w1, C = bias[0].

Strategy: data-parallel over batch across 8 NeuronCores (2 batches/core).
Host-side prep (layout only + the 0.1%-of-FLOPs rank-1 vectors):
  - a_t = (mat_0 * w2)^T per batch, cast bf16  -> [bpc, H, N]
  - b_t = mat_1^T per batch, cast bf16         -> [bpc, H, M]
  - s0 as [P, bpc*nt] column tiles; s1 = mat_1@w1 + C as [1, bpc*M] rows
    (broadcast to 128 partitions on-chip by the idle Pool engine).
Device: pure-GEMM mains psum[128n, 1024m] += a_k[h,n].T @ b_k[h,m] in bf16
(1 cycle/row on the PE array — no on-chip transposes), fused DVE epilogue
(psum + s0_col + s1_row -> bf16), bf16 stores. Host upconverts to f32.

bf16 I/O halves DMA traffic (25 MB/core vs 50) and removes the 2-cycle/row
f32 transpose tax, leaving the kernel at the PE matmul roofline. Input
loads issue half-width on the Activation HWDGE queue (finer supply at the
pipeline head) and stores on the SP queue; psum runs 4x[128,1024] banks
for eviction ILP; the final tile drains in 512-wide chunks to shorten
the tail chain.
"""

import numpy as np

import concourse.bacc as bacc
import concourse.bass as bass
import concourse.mybir as mybir
from concourse.tile import TileContext

F32 = mybir.dt.float32
BF16 = mybir.dt.bfloat16
ADD = mybir.AluOpType.add

P = 128

# Problem dims (hardcoded per contract)
B, N, M, H = 16, 2048, 2048, 512
N_CORES = 8
BPC = B // N_CORES  # batches per core


def build_program(bpc=BPC, n=N, m=M, h=H):
    kt = h // P        # contraction k-tiles (4)
    nt = n // P        # n-tiles (16)
    hw = n // 2        # half-stripe width (1024)

    nc = bacc.Bacc("TRN2", target_bir_lowering=False, debug=False)
    a_t = nc.dram_tensor("a_t", [bpc, h, n], BF16, kind="ExternalInput").ap()
    b_t = nc.dram_tensor("b_t", [bpc, h, m], BF16, kind="ExternalInput").ap()
    s0a = nc.dram_tensor("s0a", [P, bpc * nt], F32, kind="ExternalInput").ap()
    s1r = nc.dram_tensor("s1r", [1, bpc * m], F32, kind="ExternalInput").ap()
    # bf16 single-row copies of s0/s1 for the rank-1 matmul fold used by the
    # final drain chunk
    s0rb = nc.dram_tensor("s0rb", [1, bpc * n], BF16, kind="ExternalInput").ap()
    s1rb = nc.dram_tensor("s1rb", [1, bpc * m], BF16, kind="ExternalInput").ap()
    out = nc.dram_tensor("out", [bpc, n, m], BF16, kind="ExternalOutput").ap()

    with TileContext(nc) as tc:
        with (
            tc.tile_pool(name="const", bufs=1) as cpool,
            tc.tile_pool(name="ops", bufs=2) as ops,
            tc.tile_pool(name="vecs", bufs=2) as vpool,
            tc.tile_pool(name="ob", bufs=8) as obpool,
            tc.tile_pool(name="mpsum", bufs=4, space="PSUM") as mpsum,
        ):
            # PE p-state warm-up: the tensor engine ramps 0.65->1.2->2.4 GHz
            # over ~3us of continuous execution. Run throwaway matmuls on a
            # zeroed tile while the first operand stripes stream in, so real
            # matmuls start at full clock.
            warm = cpool.tile([P, 512], BF16)
            nc.vector.memset(warm, 0.0)
            ones = cpool.tile([1, 512], BF16)
            nc.gpsimd.memset(ones, 1.0)
            wp = mpsum.tile([P, hw], F32, tag="mm", name="wp")
            for _ in range(6):
                nc.tensor.matmul(
                    wp[:, :512], lhsT=warm[:, :P], rhs=warm,
                    start=True, stop=True,
                )
            for _ in range(8):
                nc.tensor.matmul(
                    wp[:, :16], lhsT=warm[:, :P], rhs=warm[:, :16],
                    start=True, stop=True,
                )

            def emit_vecs():
                s0all = cpool.tile([P, bpc * nt], F32)
                nc.sync.dma_start(out=s0all, in_=s0a)
                s1rows = cpool.tile([1, bpc * m], F32)
                nc.sync.dma_start(out=s1rows, in_=s1r)
                s0row = cpool.tile([1, bpc * n], BF16)
                nc.sync.dma_start(out=s0row, in_=s0rb)
                s1row = cpool.tile([1, bpc * m], BF16)
                nc.sync.dma_start(out=s1row, in_=s1rb)
                vpool.rank1 = (s0row, s1row)
                s1bs = []
                for bi in range(bpc):
                    s1b = vpool.tile([P, m], F32, tag=f"s1b{bi}",
                                     name=f"s1b{bi}")
                    nc.gpsimd.partition_broadcast(
                        s1b, s1rows[0:1, bi * m:(bi + 1) * m]
                    )
                    s1bs.append(s1b)
                return s0all, s1bs

            def emit_loads(bi, vecs_after_k=None):
                # supply-ordered loads, matched to the h0-first mains order:
                # k0 quarters then all k h0 pairs (PE's critical path), then
                # b h1 (needed by the h1 phase of the first tile group),
                # then a h1 (not needed until tile 8). Pieces are
                # (tile, col0, col1) spans per k.
                ak = [[] for _ in range(kt)]
                bk = [[] for _ in range(kt)]
                vecs = None

                def load(arr, dst, k, c0, c1, tag):
                    t = ops.tile([P, c1 - c0], BF16, tag=tag, name=tag)
                    nc.scalar.dma_start(
                        out=t, in_=arr[bi, k * P:(k + 1) * P, c0:c1]
                    )
                    dst[k].append((t, c0, c1))

                load(b_t, bk, 0, 0, 512, "b0q0")
                load(a_t, ak, 0, 0, 512, "a0q0")
                load(b_t, bk, 0, 512, hw, "b0q1")
                load(a_t, ak, 0, 512, hw, "a0q1")
                load(b_t, bk, 1, 0, hw, "b1h0")
                load(a_t, ak, 1, 0, hw, "a1h0")
                vecs = emit_vecs() if vecs_after_k is not None else None
                for k in range(2, kt):
                    load(b_t, bk, k, 0, hw, f"b{k}h0")
                    load(a_t, ak, k, 0, hw, f"a{k}h0")
                for k in range(kt):
                    load(b_t, bk, k, hw, m, f"b{k}h1")
                for k in range(kt):
                    load(a_t, ak, k, hw, n, f"a{k}h1")
                if vecs is not None:
                    return ak, bk, vecs
                return ak, bk

            def _piece(pieces, c0, width):
                for t, p0, p1 in pieces:
                    if p0 <= c0 and c0 + width <= p1:
                        return t[:, c0 - p0:c0 - p0 + width]
                raise AssertionError(f"no piece for {c0}+{width}")

            def lhs(ak, k, t):
                return _piece(ak[k], t * P, P)

            def rhs(bk, k, c):
                # c indexes 512-wide chunks (0..3)
                return _piece(bk[k], c * 512, 512)

            def emit_mains(bi, ak, bk, s0all, s1bs, last=False):
                s0c = s0all[:, bi * nt:(bi + 1) * nt]
                s1b = s1bs[bi]
                # group tiles in fours, h0 halves before h1, so the first
                # in-flight psum set only needs the h0 stripes (earlier PE
                # start at the pipeline head)
                order = []
                for g in range(0, nt, 4):
                    order += [(t, 0) for t in range(g, g + 4)]
                    order += [(t, 1) for t in range(g, g + 4)]
                for oi, (t, half) in enumerate(order):
                    if True:
                        if last and t == nt - 1 and half == 1:
                            # final half-tile: drain in shrinking chunks.
                            # The last chunk folds the rank-1 epilogue into
                            # two K=1 matmuls so its eviction is a plain ACT
                            # copy and its store rides the idle Pool SWDGE
                            # queue — the shortest possible tail chain.
                            s0row, s1row = vpool.rank1
                            drains = ((1024, 512, nc.sync, False),
                                      (1536, 384, nc.sync, False),
                                      (1920, 128, nc.gpsimd, True))
                            for c0, cw, eng, fold in drains:
                                mp = mpsum.tile([P, hw], F32, tag="mm",
                                                name="mp")
                                for k in range(kt):
                                    nc.tensor.matmul(
                                        mp[:, :cw],
                                        lhsT=lhs(ak, k, t),
                                        rhs=_piece(bk[k], c0, cw),
                                        start=(k == 0),
                                        stop=(k == kt - 1) and not fold,
                                    )
                                ob = obpool.tile([P, 512], BF16, tag="obl",
                                                 name="obl")
                                if fold:
                                    nc.tensor.matmul(
                                        mp[:, :cw],
                                        lhsT=s0row[0:1,
                                                   bi * n + t * P:
                                                   bi * n + (t + 1) * P],
                                        rhs=ones[0:1, :cw],
                                        start=False, stop=False,
                                    )
                                    nc.tensor.matmul(
                                        mp[:, :cw],
                                        lhsT=ones[0:1, :P],
                                        rhs=s1row[0:1,
                                                  bi * m + c0:
                                                  bi * m + c0 + cw],
                                        start=False, stop=True,
                                    )
                                    nc.scalar.copy(
                                        out=ob[:, :cw], in_=mp[:, :cw]
                                    )
                                else:
                                    nc.vector.scalar_tensor_tensor(
                                        out=ob[:, :cw],
                                        in0=mp[:, :cw],
                                        scalar=s0c[:, t:t + 1],
                                        in1=s1b[:, c0:c0 + cw],
                                        op0=ADD,
                                        op1=ADD,
                                    )
                                eng.dma_start(
                                    out=out[bi, t * P:(t + 1) * P,
                                            c0:c0 + cw],
                                    in_=ob[:, :cw],
                                )
                            continue
                        mp = mpsum.tile([P, hw], F32, tag="mm", name="mp")
                        for k in range(kt):
                            for cc in range(2):
                                c = half * 2 + cc
                                nc.tensor.matmul(
                                    mp[:, cc * 512:(cc + 1) * 512],
                                    lhsT=lhs(ak, k, t),
                                    rhs=rhs(bk, k, c),
                                    start=(k == 0),
                                    stop=(k == kt - 1),
                                )
                        ob = obpool.tile([P, hw], BF16, tag="ob", name="ob")
                        nc.vector.scalar_tensor_tensor(
                            out=ob,
                            in0=mp,
                            scalar=s0c[:, t:t + 1],
                            in1=s1b[:, half * hw:(half + 1) * hw],
                            op0=ADD,
                            op1=ADD,
                        )
                        seng = nc.sync if oi % 2 == 0 else nc.scalar
                        seng.dma_start(
                            out=out[bi, t * P:(t + 1) * P,
                                    half * hw:(half + 1) * hw],
                            in_=ob,
                        )

            ak0, bk0, (s0all, s1bs) = emit_loads(0, vecs_after_k=1)
            la = (ak0, bk0)
            for bi in range(1, bpc):
                la_next = emit_loads(bi)
                emit_mains(bi - 1, *la, s0all, s1bs)
                la = la_next
            emit_mains(bpc - 1, *la, s0all, s1bs, last=True)
    nc.compile()
    return nc


_CACHE = {}


def _get_program():
    if "nc" not in _CACHE:
        _CACHE["nc"] = build_program()
    return _CACHE["nc"]


def make_in_maps(inputs, bpc=BPC, n_cores=N_CORES, n=N, m=M, h=H):
    import ml_dtypes

    bf16 = ml_dtypes.bfloat16
    mat_0 = np.asarray(inputs["mat_0"], dtype=np.float32)
    mat_1 = np.asarray(inputs["mat_1"], dtype=np.float32)
    w = np.asarray(inputs["w"], dtype=np.float32)
    bias = np.asarray(inputs["bias"], dtype=np.float32)
    w0, w1, w2 = w[:h], w[h:2 * h], w[2 * h:]
    nt = n // P
    # host-side rank-1 epilogue vectors
    s0 = mat_0 @ w0                      # [B, n]
    s1 = mat_1 @ w1 + bias[0]            # [B, m]
    # layouts for direct DMA: pre-transposed bf16 operands
    a_t = np.ascontiguousarray(
        (mat_0 * w2).transpose(0, 2, 1)
    ).astype(bf16)                       # [B, h, n]
    b_t = np.ascontiguousarray(
        mat_1.transpose(0, 2, 1)
    ).astype(bf16)                       # [B, h, m]
    # s0 for core c: [P, bpc*nt] with batch-major columns
    s0t = np.ascontiguousarray(
        s0.reshape(-1, nt, P).transpose(0, 2, 1)              # [B, P, nt]
    )
    in_maps = []
    for c in range(n_cores):
        sl = slice(c * bpc, (c + 1) * bpc)
        s0a = np.ascontiguousarray(
            s0t[sl].transpose(1, 0, 2).reshape(P, bpc * nt)
        )
        in_maps.append(
            {
                "a_t": a_t[sl],
                "b_t": b_t[sl],
                "s0a": s0a,
                "s1r": np.ascontiguousarray(s1[sl].reshape(1, bpc * m)),
                "s0rb": np.ascontiguousarray(
                    s0[sl].reshape(1, bpc * n)).astype(bf16),
                "s1rb": np.ascontiguousarray(
                    s1[sl].reshape(1, bpc * m)).astype(bf16),
            }
        )
    return in_maps


def kernel(**inputs) -> np.ndarray:
    from concourse import bass_utils

    nc = _get_program()
    res = bass_utils.run_bass_kernel_spmd(
        nc, make_in_maps(inputs), core_ids=list(range(N_CORES))
    )
    return np.concatenate(
        [np.asarray(res.results[c]["out"]).astype(np.float32)
         for c in range(N_CORES)],
        axis=0,
    )


# revision 31
# speedup vs baseline: 1.0283x; 1.0158x over previous
"""Trainium2 Bass kernel for nn_AttentionMatrix.

Computes, for mat_0:[B,N,H], mat_1:[B,M,H], w:[3H], bias:[1]:
    out[b,n,m] = sum_h mat_0[b,n,h]*w2[h]*mat_1[b,m,h] + s0[b,n] + s1[b,m] + C
with s0 = mat_0@w0, s1 = mat_1@w1, C = bias[0].

Strategy: data-parallel over batch across 8 NeuronCores (2 batches/core).
Host-side prep (layout only + the 0.1%-of-FLOPs rank-1 vectors):
  - a_t = (mat_0 * w2)^T per batch, cast bf16  -> [bpc, H, N]
  - b_t = mat_1^T per batch, cast bf16         -> [bpc, H, M]
  - s0 as [P, bpc*nt] column tiles; s1 = mat_1@w1 + C as [1, bpc*M] rows
    (broadcast to 128 partitions on-chip by the idle Pool engine).
Device: pure-GEMM mains psum[128n, 1024m] += a_k[h,n].T @ b_k[h,m] in bf16
(1 cycle/row on the PE array — no on-chip transposes), fused DVE epilogue
(psum + s0_col + s1_row -> bf16), bf16 stores. Host upconverts to f32.

bf16 I/O halves DMA traffic (25 MB/core vs 50) and removes the 2-cycle/row
f32 transpose tax, leaving the kernel at the PE matmul roofline. Input
loads issue half-width on the Activation HWDGE queue (finer supply at the
pipeline head) and stores on the SP queue; psum runs 4x[128,1024] banks
for eviction ILP; the final tile drains in 512-wide chunks to shorten
the tail chain.
"""

import numpy as np

import concourse.bacc as bacc
import concourse.bass as bass
import concourse.mybir as mybir
from concourse.tile import TileContext

F32 = mybir.dt.float32
BF16 = mybir.dt.bfloat16
ADD = mybir.AluOpType.add

P = 128

# Problem dims (hardcoded per contract)
B, N, M, H = 16, 2048, 2048, 512
N_CORES = 8
BPC = B // N_CORES  # batches per core


def build_program(bpc=BPC, n=N, m=M, h=H):
    kt = h // P        # contraction k-tiles (4)
    nt = n // P        # n-tiles (16)
    hw = n // 2        # half-stripe width (1024)

    nc = bacc.Bacc("TRN2", target_bir_lowering=False, debug=False)
    a_t = nc.dram_tensor("a_t", [bpc, h, n], BF16, kind="ExternalInput").ap()
    b_t = nc.dram_tensor("b_t", [bpc, h, m], BF16, kind="ExternalInput").ap()
    s0a = nc.dram_tensor("s0a", [P, bpc * nt], F32, kind="ExternalInput").ap()
    s1r = nc.dram_tensor("s1r", [1, bpc * m], F32, kind="ExternalInput").ap()
    # bf16 single-row copies of s0/s1 for the rank-1 matmul fold used by the
    # final drain chunk
    s0rb = nc.dram_tensor("s0rb", [1, bpc * n], BF16, kind="ExternalInput").ap()
    s1rb = nc.dram_tensor("s1rb", [1, bpc * m], BF16, kind="ExternalInput").ap()
    out = nc.dram_tensor("out", [bpc, n, m], BF16, kind="ExternalOutput").ap()

    with TileContext(nc) as tc:
        with (
            tc.tile_pool(name="const", bufs=1) as cpool,
            tc.tile_pool(name="ops", bufs=2) as ops,
            tc.tile_pool(name="vecs", bufs=2) as vpool,
            tc.tile_pool(name="ob", bufs=8) as obpool,
            tc.tile_pool(name="mpsum", bufs=4, space="PSUM") as mpsum,
        ):
            # PE p-state warm-up: the tensor engine ramps 0.65->1.2->2.4 GHz
            # over ~3us of continuous execution. Run throwaway matmuls on a
            # zeroed tile while the first operand stripes stream in, so real
            # matmuls start at full clock.
            warm = cpool.tile([P, 512], BF16)
            nc.vector.memset(warm, 0.0)
            ones = cpool.tile([1, 512], BF16)
            nc.gpsimd.memset(ones, 1.0)
            wp = mpsum.tile([P, hw], F32, tag="mm", name="wp")
            for _ in range(6):
                nc.tensor.matmul(
                    wp[:, :512], lhsT=warm[:, :P], rhs=warm,
                    start=True, stop=True,
                )
            for _ in range(40):
                nc.tensor.matmul(
                    wp[:, :16], lhsT=warm[:, :P], rhs=warm[:, :16],
                    start=True, stop=True,
                )

            def emit_vecs():
                # all vector loads ride the Pool SWDGE queue: zero pressure
                # on the shared HWDGE issue pipeline that feeds the stripe
                # loads and stores
                s0all = cpool.tile([P, bpc * nt], F32)
                nc.gpsimd.dma_start(out=s0all, in_=s0a)
                s1rows = cpool.tile([1, bpc * m], F32)
                nc.gpsimd.dma_start(out=s1rows, in_=s1r)
                s1bs = []
                for bi in range(bpc):
                    s1b = vpool.tile([P, m], F32, tag=f"s1b{bi}",
                                     name=f"s1b{bi}")
                    nc.gpsimd.partition_broadcast(
                        s1b, s1rows[0:1, bi * m:(bi + 1) * m]
                    )
                    s1bs.append(s1b)
                s0row = cpool.tile([1, bpc * n], BF16)
                nc.gpsimd.dma_start(out=s0row, in_=s0rb)
                s1row = cpool.tile([1, bpc * m], BF16)
                nc.gpsimd.dma_start(out=s1row, in_=s1rb)
                vpool.rank1 = (s0row, s1row)
                return s0all, s1bs

            def emit_loads(bi, vecs_after_k=None):
                # supply-ordered loads, matched to the h0-first mains order:
                # k0 quarters then all k h0 pairs (PE's critical path), then
                # b h1 (needed by the h1 phase of the first tile group),
                # then a h1 (not needed until tile 8). Pieces are
                # (tile, col0, col1) spans per k.
                ak = [[] for _ in range(kt)]
                bk = [[] for _ in range(kt)]
                vecs = None

                def load(arr, dst, k, c0, c1, tag):
                    t = ops.tile([P, c1 - c0], BF16, tag=tag, name=tag)
                    nc.scalar.dma_start(
                        out=t, in_=arr[bi, k * P:(k + 1) * P, c0:c1]
                    )
                    dst[k].append((t, c0, c1))

                vecs = emit_vecs() if vecs_after_k is not None else None
                for k in range(kt):
                    load(b_t, bk, k, 0, hw, f"b{k}h0")
                    load(a_t, ak, k, 0, hw, f"a{k}h0")
                for k in range(kt):
                    load(b_t, bk, k, hw, m, f"b{k}h1")
                for k in range(kt):
                    load(a_t, ak, k, hw, n, f"a{k}h1")
                if vecs is not None:
                    return ak, bk, vecs
                return ak, bk

            def _piece(pieces, c0, width):
                for t, p0, p1 in pieces:
                    if p0 <= c0 and c0 + width <= p1:
                        return t[:, c0 - p0:c0 - p0 + width]
                raise AssertionError(f"no piece for {c0}+{width}")

            def lhs(ak, k, t):
                return _piece(ak[k], t * P, P)

            def rhs(bk, k, c):
                # c indexes 512-wide chunks (0..3)
                return _piece(bk[k], c * 512, 512)

            def emit_mains(bi, ak, bk, s0all, s1bs, last=False):
                s0c = s0all[:, bi * nt:(bi + 1) * nt]
                s1b = s1bs[bi]
                # group tiles in fours, h0 halves before h1, so the first
                # in-flight psum set only needs the h0 stripes (earlier PE
                # start at the pipeline head)
                order = []
                for g in range(0, nt, 4):
                    order += [(t, 0) for t in range(g, g + 4)]
                    order += [(t, 1) for t in range(g, g + 4)]
                for oi, (t, half) in enumerate(order):
                    if True:
                        if last and t == nt - 1 and half == 1:
                            # final half-tile: drain in shrinking chunks.
                            # The last chunk folds the rank-1 epilogue into
                            # two K=1 matmuls so its eviction is a plain ACT
                            # copy and its store rides the idle Pool SWDGE
                            # queue — the shortest possible tail chain.
                            s0row, s1row = vpool.rank1
                            drains = ((1024, 512, nc.sync, False),
                                      (1536, 384, nc.sync, False),
                                      (1920, 128, nc.gpsimd, True))
                            for c0, cw, eng, fold in drains:
                                mp = mpsum.tile([P, hw], F32, tag="mm",
                                                name="mp")
                                for k in range(kt):
                                    nc.tensor.matmul(
                                        mp[:, :cw],
                                        lhsT=lhs(ak, k, t),
                                        rhs=_piece(bk[k], c0, cw),
                                        start=(k == 0),
                                        stop=(k == kt - 1) and not fold,
                                    )
                                ob = obpool.tile([P, 512], BF16, tag="obl",
                                                 name="obl")
                                if fold:
                                    nc.tensor.matmul(
                                        mp[:, :cw],
                                        lhsT=s0row[0:1,
                                                   bi * n + t * P:
                                                   bi * n + (t + 1) * P],
                                        rhs=ones[0:1, :cw],
                                        start=False, stop=False,
                                    )
                                    nc.tensor.matmul(
                                        mp[:, :cw],
                                        lhsT=ones[0:1, :P],
                                        rhs=s1row[0:1,
                                                  bi * m + c0:
                                                  bi * m + c0 + cw],
                                        start=False, stop=True,
                                    )
                                    nc.scalar.copy(
                                        out=ob[:, :cw], in_=mp[:, :cw]
                                    )
                                else:
                                    nc.vector.scalar_tensor_tensor(
                                        out=ob[:, :cw],
                                        in0=mp[:, :cw],
                                        scalar=s0c[:, t:t + 1],
                                        in1=s1b[:, c0:c0 + cw],
                                        op0=ADD,
                                        op1=ADD,
                                    )
                                eng.dma_start(
                                    out=out[bi, t * P:(t + 1) * P,
                                            c0:c0 + cw],
                                    in_=ob[:, :cw],
                                )
                            continue
                        mp = mpsum.tile([P, hw], F32, tag="mm", name="mp")
                        for k in range(kt):
                            for cc in range(2):
                                c = half * 2 + cc
                                nc.tensor.matmul(
                                    mp[:, cc * 512:(cc + 1) * 512],
                                    lhsT=lhs(ak, k, t),
                                    rhs=rhs(bk, k, c),
                                    start=(k == 0),
                                    stop=(k == kt - 1),
                                )
                        ob = obpool.tile([P, hw], BF16, tag="ob", name="ob")
                        nc.vector.scalar_tensor_tensor(
                            out=ob,
                            in0=mp,
                            scalar=s0c[:, t:t + 1],
                            in1=s1b[:, half * hw:(half + 1) * hw],
                            op0=ADD,
                            op1=ADD,
                        )
                        seng = nc.sync if oi % 2 == 0 else nc.scalar
                        seng.dma_start(
                            out=out[bi, t * P:(t + 1) * P,
                                    half * hw:(half + 1) * hw],
                            in_=ob,
                        )

            ak0, bk0, (s0all, s1bs) = emit_loads(0, vecs_after_k=1)
            la = (ak0, bk0)
            for bi in range(1, bpc):
                la_next = emit_loads(bi)
                emit_mains(bi - 1, *la, s0all, s1bs)
                la = la_next
            emit_mains(bpc - 1, *la, s0all, s1bs, last=True)
    nc.compile()
    return nc


_CACHE = {}


def _get_program():
    if "nc" not in _CACHE:
        _CACHE["nc"] = build_program()
    return _CACHE["nc"]


def make_in_maps(inputs, bpc=BPC, n_cores=N_CORES, n=N, m=M, h=H):
    import ml_dtypes

    bf16 = ml_dtypes.bfloat16
    mat_0 = np.asarray(inputs["mat_0"], dtype=np.float32)
    mat_1 = np.asarray(inputs["mat_1"], dtype=np.float32)
    w = np.asarray(inputs["w"], dtype=np.float32)
    bias = np.asarray(inputs["bias"], dtype=np.float32)
    w0, w1, w2 = w[:h], w[h:2 * h], w[2 * h:]
    nt = n // P
    # host-side rank-1 epilogue vectors
    s0 = mat_0 @ w0                      # [B, n]
    s1 = mat_1 @ w1 + bias[0]            # [B, m]
    # layouts for direct DMA: pre-transposed bf16 operands
    a_t = np.ascontiguousarray(
        (mat_0 * w2).transpose(0, 2, 1)
    ).astype(bf16)                       # [B, h, n]
    b_t = np.ascontiguousarray(
        mat_1.transpose(0, 2, 1)
    ).astype(bf16)                       # [B, h, m]
    # s0 for core c: [P, bpc*nt] with batch-major columns
    s0t = np.ascontiguousarray(
        s0.reshape(-1, nt, P).transpose(0, 2, 1)              # [B, P, nt]
    )
    in_maps = []
    for c in range(n_cores):
        sl = slice(c * bpc, (c + 1) * bpc)
        s0a = np.ascontiguousarray(
            s0t[sl].transpose(1, 0, 2).reshape(P, bpc * nt)
        )
        in_maps.append(
            {
                "a_t": a_t[sl],
                "b_t": b_t[sl],
                "s0a": s0a,
                "s1r": np.ascontiguousarray(s1[sl].reshape(1, bpc * m)),
                "s0rb": np.ascontiguousarray(
                    s0[sl].reshape(1, bpc * n)).astype(bf16),
                "s1rb": np.ascontiguousarray(
                    s1[sl].reshape(1, bpc * m)).astype(bf16),
            }
        )
    return in_maps


def kernel(**inputs) -> np.ndarray:
    from concourse import bass_utils

    nc = _get_program()
    res = bass_utils.run_bass_kernel_spmd(
        nc, make_in_maps(inputs), core_ids=list(range(N_CORES))
    )
    return np.concatenate(
        [np.asarray(res.results[c]["out"]).astype(np.float32)
         for c in range(N_CORES)],
        axis=0,
    )
